# revision 1
# baseline (speedup 1.0000x reference)
"""DEMA (double exponential moving average) Trainium2 kernel.

Problem: x [32, 2048, 512] f32 -> (res = x - ma, ma) where ma is the DEMA
scan over the time axis (alpha = beta = 0.3).

Approach: the 2-state linear recurrence has constant coefficients, so
ma[t] is a causal convolution of x with the impulse response h[d] =
(A^d c)[0] plus an initial-state term.  |eig(A)| = sqrt(0.7) ~ 0.8367, so
h decays below 4e-11 by d = 128: a 128-tap truncated convolution is exact
to fp32 precision.  Per 128-step time chunk the outputs are
    ma_chunk[i] = T0 @ x_chunk[i] + T1 @ x_chunk[i-1]
with lower/upper-triangular Toeplitz matrices T0/T1 (and an exact
special-cased first-chunk matrix TF that folds in the initial state).
These run as fp32 matmuls on the tensor engine with time on the
contraction axis; (batch x channel) rides the free axis.

Sharding: fully data-parallel over batch, 4 batches per core x 8 cores.
"""

import numpy as np

ALPHA = 0.3
BETA = 0.3
B, T, C = 32, 2048, 512
N_CORES = 8
B_LOCAL = B // N_CORES  # 4
L = 128                 # chunk length == conv taps
N_CHUNKS = T // L       # 16


def _build_matrices():
    A = np.array([[1 - ALPHA, 1 - ALPHA],
                  [-ALPHA * BETA, 1 - ALPHA * BETA]], dtype=np.float64)
    c = np.array([ALPHA, ALPHA * BETA], dtype=np.float64)

    # impulse response h[d] = (A^d c)[0], d = 0..2L-1
    hh = np.zeros(2 * L)
    v = c.copy()
    for d in range(2 * L):
        hh[d] = v[0]
        v = A @ v

    # initial-state response p[j], q[j] = (A^j)[0, :]
    p = np.zeros(L)
    q = np.zeros(L)
    M = np.eye(2)
    for j in range(L):
        p[j] = M[0, 0]
        q[j] = M[0, 1]
        M = A @ M

    T0 = np.zeros((L, L))
    for j in range(L):
        T0[j, :j + 1] = hh[j::-1]          # T0[j, k] = h[j - k], k <= j
    T1 = np.zeros((L, L))
    for j in range(L):
        for k in range(j + 1, L):
            T1[j, k] = hh[L + j - k]       # cross-chunk taps, distance < L
    TF = T0.copy()                          # first chunk: exact init state
    TF[0, :] = 0.0
    TF[0, 0] = 1.0                          # ma[0] = x[0]
    for j in range(1, L):
        TF[j, 0] = p[j] - q[j]             # coeff on x[0]
        TF[j, 1] = hh[j - 1] + q[j]        # coeff on x[1]

    # matmul computes lhsT.T @ rhs -> pass the transpose as the stationary op
    to32 = lambda m: np.ascontiguousarray(m.T, dtype=np.float32)
    return to32(T0), to32(T1), to32(TF)


_NC_CACHE = {}


def _build_nc(n_iter=1):
    if n_iter in _NC_CACHE:
        return _NC_CACHE[n_iter]

    import concourse.bacc as bacc
    import concourse.mybir as mybir
    import concourse.tile as tile

    f32 = mybir.dt.float32
    nc = bacc.Bacc("TRN2", target_bir_lowering=False, debug=False)

    x = nc.dram_tensor("x", [B_LOCAL, T, C], f32, kind="ExternalInput")
    res = nc.dram_tensor("res", [B_LOCAL, T, C], f32, kind="ExternalOutput")
    ma = nc.dram_tensor("ma", [B_LOCAL, T, C], f32, kind="ExternalOutput")

    w0t_np, w1t_np, wft_np = _build_matrices()
    w0d = nc.inline_tensor(w0t_np, name="w0T")
    w1d = nc.inline_tensor(w1t_np, name="w1T")
    wfd = nc.inline_tensor(wft_np, name="wfT")

    xap = x.ap()
    res_ap = res.ap()
    ma_ap = ma.ap()

    def chunk_dram(ap, i):
        # [B_LOCAL, L, C] slice viewed as [L(time, partitions), B_LOCAL, C]
        return ap[:, i * L:(i + 1) * L, :].rearrange("b t c -> t b c")

    with tile.TileContext(nc) as tc:
        with (
            tc.tile_pool(name="weights", bufs=1) as wpool,
            tc.tile_pool(name="xin", bufs=6) as xpool,
            tc.tile_pool(name="maout", bufs=4) as mapool,
            tc.tile_pool(name="resout", bufs=4) as respool,
            tc.tile_pool(name="psum", bufs=4, space="PSUM") as pspool,
        ):
            w0 = wpool.tile([L, L], f32, tag="w0")
            nc.sync.dma_start(w0[:], w0d[:])
            w1 = wpool.tile([L, L], f32, tag="w1")
            nc.sync.dma_start(w1[:], w1d[:])
            wf = wpool.tile([L, L], f32, tag="wf")
            nc.sync.dma_start(wf[:], wfd[:])

            for _rep in range(n_iter):
                x_prev = None
                for i in range(N_CHUNKS):
                    xt = xpool.tile([L, B_LOCAL, C], f32, tag="x")
                    nc.sync.dma_start(xt[:], chunk_dram(xap, i))

                    ma_t = mapool.tile([L, B_LOCAL, C], f32, tag="ma")
                    res_t = respool.tile([L, B_LOCAL, C], f32, tag="res")
                    # 2-bank PSUM tiles; both PSUM-reading ops on DVE at
                    # [128, 1024] granularity (ACT fp32 copy is ~2x slower and
                    # stalls PSUM-bank recycling).
                    for g in range(B_LOCAL // 2):
                        ps = pspool.tile([L, 2, C], f32, tag="ps")
                        for k in range(2):
                            nb = 2 * g + k
                            if i == 0:
                                nc.tensor.matmul(ps[:, k, :], wf[:], xt[:, nb, :],
                                                 start=True, stop=True)
                            else:
                                nc.tensor.matmul(ps[:, k, :], w1[:],
                                                 x_prev[:, nb, :],
                                                 start=True, stop=False)
                                nc.tensor.matmul(ps[:, k, :], w0[:], xt[:, nb, :],
                                                 start=False, stop=True)
                        bsl = slice(2 * g, 2 * g + 2)
                        psf = ps[:].rearrange("t k c -> t (k c)")
                        nc.vector.tensor_copy(
                            ma_t[:, bsl, :].rearrange("t k c -> t (k c)"), psf)
                        nc.vector.tensor_sub(
                            res_t[:, bsl, :].rearrange("t k c -> t (k c)"),
                            xt[:, bsl, :].rearrange("t k c -> t (k c)"), psf)

                    # out-DMAs issue from the ACT HWDGE queue: their sem waits
                    # would head-of-line-block the SP queue's input DMAs
                    nc.scalar.dma_start(chunk_dram(ma_ap, i), ma_t[:])
                    nc.scalar.dma_start(chunk_dram(res_ap, i), res_t[:])
                    x_prev = xt

    nc.compile()
    _NC_CACHE[n_iter] = nc
    return nc


def kernel(x):
    x = np.ascontiguousarray(np.asarray(x), dtype=np.float32)
    assert x.shape == (B, T, C), x.shape

    from concourse import bass_utils

    nc = _build_nc()
    in_maps = [{"x": x[i * B_LOCAL:(i + 1) * B_LOCAL]} for i in range(N_CORES)]
    out = bass_utils.run_bass_kernel_spmd(nc, in_maps, core_ids=list(range(N_CORES)))
    res = np.concatenate([out.results[i]["res"] for i in range(N_CORES)], axis=0)
    ma = np.concatenate([out.results[i]["ma"] for i in range(N_CORES)], axis=0)
    return res, ma



# revision 2
# speedup vs baseline: 5.2524x; 5.2524x over previous
"""DEMA (double exponential moving average) Trainium2 kernel.

Problem: x [32, 2048, 512] f32 -> (res = x - ma, ma) where ma is the DEMA
scan over the time axis (alpha = beta = 0.3).

Approach: the 2-state linear recurrence has constant coefficients, so
ma[t] is a causal convolution of x with the impulse response h[d] =
(A^d c)[0] plus an initial-state term.  |eig(A)| = sqrt(0.7) ~ 0.8367, so
h decays below 4e-11 by d = 128: a 128-tap truncated convolution is exact
to fp32 precision.  Per 128-step time chunk the outputs are
    ma_chunk[i] = T0 @ x_chunk[i] + T1 @ x_chunk[i-1]
with lower/upper-triangular Toeplitz matrices T0/T1 (and an exact
special-cased first-chunk matrix TF that folds in the initial state).
These run as fp32 matmuls on the tensor engine with time on the
contraction axis; the channel axis rides the free axis.

DMA layout: this kernel is HBM-bandwidth-bound (48 MB/core must move;
the per-NeuronCore HBM limit is ~358 GB/s -> ~140 us floor).  Measured
on HW: the b-interleaved [t, b, c] chunk DMAs of the naive layout reach
only ~310 GB/s because each descriptor covers just 2 KB and consecutive
descriptors hop between 4-MB-apart batch rows.  Transfers that cover a
1-MB *sequential* DRAM span per DMA (4 time chunks of one batch row,
tile [t, j, c]) reach ~355 GB/s, within 1% of the per-core roofline.

Sharding: fully data-parallel over batch, 4 batches per core x 8 cores.
"""

import numpy as np

ALPHA = 0.3
BETA = 0.3
B, T, C = 32, 2048, 512
N_CORES = 8
B_LOCAL = B // N_CORES  # 4
L = 128                 # chunk length == conv taps
N_CHUNKS = T // L       # 16
NB = 4                  # chunks per DMA macro-tile (1 MB sequential span)
N_MACRO = N_CHUNKS // NB


def _build_matrices():
    A = np.array([[1 - ALPHA, 1 - ALPHA],
                  [-ALPHA * BETA, 1 - ALPHA * BETA]], dtype=np.float64)
    c = np.array([ALPHA, ALPHA * BETA], dtype=np.float64)

    # impulse response h[d] = (A^d c)[0], d = 0..2L-1
    hh = np.zeros(2 * L)
    v = c.copy()
    for d in range(2 * L):
        hh[d] = v[0]
        v = A @ v

    # initial-state response p[j], q[j] = (A^j)[0, :]
    p = np.zeros(L)
    q = np.zeros(L)
    M = np.eye(2)
    for j in range(L):
        p[j] = M[0, 0]
        q[j] = M[0, 1]
        M = A @ M

    T0 = np.zeros((L, L))
    for j in range(L):
        T0[j, :j + 1] = hh[j::-1]          # T0[j, k] = h[j - k], k <= j
    T1 = np.zeros((L, L))
    for j in range(L):
        for k in range(j + 1, L):
            T1[j, k] = hh[L + j - k]       # cross-chunk taps, distance < L
    TF = T0.copy()                          # first chunk: exact init state
    TF[0, :] = 0.0
    TF[0, 0] = 1.0                          # ma[0] = x[0]
    for j in range(1, L):
        TF[j, 0] = p[j] - q[j]             # coeff on x[0]
        TF[j, 1] = hh[j - 1] + q[j]        # coeff on x[1]

    # matmul computes lhsT.T @ rhs -> pass the transpose as the stationary op
    to32 = lambda m: np.ascontiguousarray(m.T, dtype=np.float32)
    return to32(T0), to32(T1), to32(TF)


_NC_CACHE = {}


def _build_nc(n_iter=1):
    if n_iter in _NC_CACHE:
        return _NC_CACHE[n_iter]

    import concourse.bacc as bacc
    import concourse.mybir as mybir
    import concourse.tile as tile

    f32 = mybir.dt.float32
    nc = bacc.Bacc("TRN2", target_bir_lowering=False, debug=False)

    x = nc.dram_tensor("x", [B_LOCAL, T, C], f32, kind="ExternalInput")
    res = nc.dram_tensor("res", [B_LOCAL, T, C], f32, kind="ExternalOutput")
    ma = nc.dram_tensor("ma", [B_LOCAL, T, C], f32, kind="ExternalOutput")

    w0t_np, w1t_np, wft_np = _build_matrices()
    w0d = nc.inline_tensor(w0t_np, name="w0T")
    w1d = nc.inline_tensor(w1t_np, name="w1T")
    wfd = nc.inline_tensor(wft_np, name="wfT")

    xap = x.ap()
    res_ap = res.ap()
    ma_ap = ma.ap()

    def bspan(ap, b, i):
        # one batch row's NB consecutive time chunks: a 1 MB sequential DRAM
        # span viewed as [L(time, partitions), NB(chunk), C]
        return ap[b, i * NB * L:(i + 1) * NB * L, :].rearrange(
            "(j t) c -> t j c", t=L)

    with tile.TileContext(nc) as tc:
        with (
            tc.tile_pool(name="weights", bufs=1) as wpool,
            tc.tile_pool(name="xin", bufs=3) as xpool,
            tc.tile_pool(name="maout", bufs=4) as mapool,
            tc.tile_pool(name="resout", bufs=4) as respool,
            tc.tile_pool(name="psum", bufs=4, space="PSUM") as pspool,
        ):
            w0 = wpool.tile([L, L], f32, tag="w0")
            nc.sync.dma_start(w0[:], w0d[:])
            w1 = wpool.tile([L, L], f32, tag="w1")
            nc.sync.dma_start(w1[:], w1d[:])
            wf = wpool.tile([L, L], f32, tag="wf")
            nc.sync.dma_start(wf[:], wfd[:])

            for _rep in range(n_iter):
                x_prev = [None] * B_LOCAL
                for i in range(N_MACRO):
                    for b in range(B_LOCAL):
                        xt = xpool.tile([L, NB, C], f32, tag=f"x{b}")
                        nc.sync.dma_start(xt[:], bspan(xap, b, i))

                        ma_t = mapool.tile([L, NB, C], f32, tag="ma")
                        res_t = respool.tile([L, NB, C], f32, tag="res")
                        # 2-bank PSUM tiles; both PSUM-reading ops on DVE at
                        # [128, 1024] granularity (ACT fp32 copy is ~2x
                        # slower and stalls PSUM-bank recycling).
                        for g in range(NB // 2):
                            ps = pspool.tile([L, 2, C], f32, tag="ps")
                            for k in range(2):
                                j = 2 * g + k
                                if i == 0 and j == 0:
                                    nc.tensor.matmul(ps[:, k, :], wf[:],
                                                     xt[:, 0, :],
                                                     start=True, stop=True)
                                else:
                                    xp, jp = ((xt, j - 1) if j > 0
                                              else (x_prev[b], NB - 1))
                                    nc.tensor.matmul(ps[:, k, :], w1[:],
                                                     xp[:, jp, :],
                                                     start=True, stop=False)
                                    nc.tensor.matmul(ps[:, k, :], w0[:],
                                                     xt[:, j, :],
                                                     start=False, stop=True)
                            jsl = slice(2 * g, 2 * g + 2)
                            psf = ps[:].rearrange("t k c -> t (k c)")
                            nc.vector.tensor_copy(
                                ma_t[:, jsl, :].rearrange("t k c -> t (k c)"),
                                psf)
                            nc.vector.tensor_sub(
                                res_t[:, jsl, :].rearrange("t k c -> t (k c)"),
                                xt[:, jsl, :].rearrange("t k c -> t (k c)"),
                                psf)

                        # out-DMAs on the ACT HWDGE queue: their sem waits
                        # would head-of-line-block the SP queue's input DMAs
                        nc.scalar.dma_start(bspan(ma_ap, b, i), ma_t[:])
                        nc.scalar.dma_start(bspan(res_ap, b, i), res_t[:])
                        x_prev[b] = xt

    nc.compile()
    _NC_CACHE[n_iter] = nc
    return nc


def kernel(x):
    x = np.ascontiguousarray(np.asarray(x), dtype=np.float32)
    assert x.shape == (B, T, C), x.shape

    from concourse import bass_utils

    nc = _build_nc()
    in_maps = [{"x": x[i * B_LOCAL:(i + 1) * B_LOCAL]} for i in range(N_CORES)]
    out = bass_utils.run_bass_kernel_spmd(nc, in_maps, core_ids=list(range(N_CORES)))
    res = np.concatenate([out.results[i]["res"] for i in range(N_CORES)], axis=0)
    ma = np.concatenate([out.results[i]["ma"] for i in range(N_CORES)], axis=0)
    return res, ma


# revision 3
# speedup vs baseline: 10.2227x; 1.9463x over previous
"""DEMA (double exponential moving average) Trainium2 kernel.

Problem: x [32, 2048, 512] f32 -> (res = x - ma, ma) where ma is the DEMA
scan over the time axis (alpha = beta = 0.3).

Math: the 2-state linear recurrence has constant coefficients, so ma[t]
is a causal convolution of x with the impulse response h[d] = (A^d c)[0]
plus an initial-state term.  |eig(A)| = sqrt(0.7) ~ 0.8367, so h decays
below 4e-11 by d = 128: a 128-tap truncated convolution is exact to
working precision.  Per 128-step time chunk:
    ma_chunk[i] = T0 @ x_chunk[i] + T1 @ x_chunk[i-1]
with lower/upper-triangular Toeplitz matrices T0/T1 (and an exact
special-cased first-chunk matrix TF that folds in the initial state).
These run as matmuls on the tensor engine with time on the contraction
axis (partitions); channels ride the free axis.

Precision/bandwidth tradeoff: the kernel is HBM-bandwidth-bound, and the
target tolerance (rel_err < 2e-2) leaves ~5x headroom over bf16
quantization (measured end-to-end max_rel ~ 6e-3).  So x is downcast to
bf16 on the host, the device moves bf16 in/out (24 MB/core instead of
48 MB), accumulates matmuls in fp32 PSUM, and outputs are upcast to f32
on the host after the gather.  HBM floor: 24 MB / ~358 GB/s ~ 70 us.

DMA layout: transfers must cover long *sequential* DRAM spans per DMA to
reach the per-NC HBM roofline (b-interleaved chunk layouts measured ~13%
slower).  Each DMA covers NB consecutive time chunks of one batch row,
tile [t=128, j=NB, c=512].

Sharding: fully data-parallel over batch, 4 batches per core x 8 cores.
"""

import numpy as np
import ml_dtypes

ALPHA = 0.3
BETA = 0.3
B, T, C = 32, 2048, 512
N_CORES = 8
B_LOCAL = B // N_CORES  # 4
L = 128                 # chunk length == conv taps
N_CHUNKS = T // L       # 16
NB = 8                  # chunks per DMA macro-tile (1 MB bf16 span)

BF16 = ml_dtypes.bfloat16


def _build_matrices():
    A = np.array([[1 - ALPHA, 1 - ALPHA],
                  [-ALPHA * BETA, 1 - ALPHA * BETA]], dtype=np.float64)
    c = np.array([ALPHA, ALPHA * BETA], dtype=np.float64)

    # impulse response h[d] = (A^d c)[0], d = 0..2L-1
    hh = np.zeros(2 * L)
    v = c.copy()
    for d in range(2 * L):
        hh[d] = v[0]
        v = A @ v

    # initial-state response p[j], q[j] = (A^j)[0, :]
    p = np.zeros(L)
    q = np.zeros(L)
    M = np.eye(2)
    for j in range(L):
        p[j] = M[0, 0]
        q[j] = M[0, 1]
        M = A @ M

    T0 = np.zeros((L, L))
    for j in range(L):
        T0[j, :j + 1] = hh[j::-1]          # T0[j, k] = h[j - k], k <= j
    T1 = np.zeros((L, L))
    for j in range(L):
        for k in range(j + 1, L):
            T1[j, k] = hh[L + j - k]       # cross-chunk taps, distance < L
    TF = T0.copy()                          # first chunk: exact init state
    TF[0, :] = 0.0
    TF[0, 0] = 1.0                          # ma[0] = x[0]
    for j in range(1, L):
        TF[j, 0] = p[j] - q[j]             # coeff on x[0]
        TF[j, 1] = hh[j - 1] + q[j]        # coeff on x[1]

    # matmul computes lhsT.T @ rhs -> pass the transpose as the stationary op
    tobf = lambda m: np.ascontiguousarray(m.T).astype(BF16)
    return tobf(T0), tobf(T1), tobf(TF)


_NC_CACHE = {}


def _build_nc(n_iter=1, nb=NB):
    key = (n_iter, nb)
    if key in _NC_CACHE:
        return _NC_CACHE[key]

    import concourse.bacc as bacc
    import concourse.mybir as mybir
    import concourse.tile as tile

    bf16 = mybir.dt.bfloat16
    f32 = mybir.dt.float32
    nc = bacc.Bacc("TRN2", target_bir_lowering=False, debug=False)

    x = nc.dram_tensor("x", [B_LOCAL, T, C], bf16, kind="ExternalInput")
    res = nc.dram_tensor("res", [B_LOCAL, T, C], bf16, kind="ExternalOutput")
    ma = nc.dram_tensor("ma", [B_LOCAL, T, C], bf16, kind="ExternalOutput")

    w0t_np, w1t_np, wft_np = _build_matrices()
    w0d = nc.inline_tensor(w0t_np, name="w0T")
    w1d = nc.inline_tensor(w1t_np, name="w1T")
    wfd = nc.inline_tensor(wft_np, name="wfT")

    xap = x.ap()
    res_ap = res.ap()
    ma_ap = ma.ap()
    n_macro = N_CHUNKS // nb

    def bspan(ap, b, i):
        # one batch row's nb consecutive time chunks: a sequential DRAM span
        # viewed as [L(time, partitions), nb(chunk), C]
        return ap[b, i * nb * L:(i + 1) * nb * L, :].rearrange(
            "(j t) c -> t j c", t=L)

    with tile.TileContext(nc) as tc:
        with (
            tc.tile_pool(name="weights", bufs=1) as wpool,
            tc.tile_pool(name="xin", bufs=3) as xpool,
            tc.tile_pool(name="maout", bufs=4) as mapool,
            tc.tile_pool(name="resout", bufs=4) as respool,
            tc.tile_pool(name="psum", bufs=4, space="PSUM") as pspool,
        ):
            w0 = wpool.tile([L, L], bf16, tag="w0")
            nc.sync.dma_start(w0[:], w0d[:])
            w1 = wpool.tile([L, L], bf16, tag="w1")
            nc.sync.dma_start(w1[:], w1d[:])
            wf = wpool.tile([L, L], bf16, tag="wf")
            nc.sync.dma_start(wf[:], wfd[:])

            for _rep in range(n_iter):
                x_prev = [None] * B_LOCAL
                for i in range(n_macro):
                    for b in range(B_LOCAL):
                        xt = xpool.tile([L, nb, C], bf16, tag=f"x{b}")
                        nc.sync.dma_start(xt[:], bspan(xap, b, i))

                        ma_t = mapool.tile([L, nb, C], bf16, tag="ma")
                        res_t = respool.tile([L, nb, C], bf16, tag="res")
                        # 2-bank PSUM tiles; PSUM-reading copy on DVE at
                        # [128, 1024] granularity; the sub then runs
                        # all-SBUF bf16 (2x DVE mode eligible).
                        for g in range(nb // 2):
                            ps = pspool.tile([L, 2, C], f32, tag="ps")
                            for k in range(2):
                                j = 2 * g + k
                                if i == 0 and j == 0:
                                    nc.tensor.matmul(ps[:, k, :], wf[:],
                                                     xt[:, 0, :],
                                                     start=True, stop=True)
                                else:
                                    xp, jp = ((xt, j - 1) if j > 0
                                              else (x_prev[b], nb - 1))
                                    nc.tensor.matmul(ps[:, k, :], w1[:],
                                                     xp[:, jp, :],
                                                     start=True, stop=False)
                                    nc.tensor.matmul(ps[:, k, :], w0[:],
                                                     xt[:, j, :],
                                                     start=False, stop=True)
                            jsl = slice(2 * g, 2 * g + 2)
                            flat = lambda ap_: ap_.rearrange("t k c -> t (k c)")
                            nc.vector.tensor_copy(
                                flat(ma_t[:, jsl, :]), flat(ps[:]))
                            nc.vector.tensor_sub(
                                flat(res_t[:, jsl, :]),
                                flat(xt[:, jsl, :]),
                                flat(ma_t[:, jsl, :]))

                        # out-DMAs on the ACT HWDGE queue: their sem waits
                        # would head-of-line-block the SP queue's input DMAs
                        nc.scalar.dma_start(bspan(ma_ap, b, i), ma_t[:])
                        nc.scalar.dma_start(bspan(res_ap, b, i), res_t[:])
                        x_prev[b] = xt

    nc.compile()
    _NC_CACHE[key] = nc
    return nc


def kernel(x):
    x = np.asarray(x)
    assert x.shape == (B, T, C), x.shape
    xb = np.ascontiguousarray(x.astype(BF16))

    from concourse import bass_utils

    nc = _build_nc()
    in_maps = [{"x": xb[i * B_LOCAL:(i + 1) * B_LOCAL]} for i in range(N_CORES)]
    out = bass_utils.run_bass_kernel_spmd(nc, in_maps, core_ids=list(range(N_CORES)))
    res = np.concatenate(
        [np.asarray(out.results[i]["res"]).astype(np.float32)
         for i in range(N_CORES)], axis=0)
    ma = np.concatenate(
        [np.asarray(out.results[i]["ma"]).astype(np.float32)
         for i in range(N_CORES)], axis=0)
    return res, ma


# revision 4
# speedup vs baseline: 10.6983x; 1.0465x over previous
"""DEMA (double exponential moving average) Trainium2 kernel.

Problem: x [32, 2048, 512] f32 -> (res = x - ma, ma) where ma is the DEMA
scan over the time axis (alpha = beta = 0.3).

Math: the 2-state linear recurrence has constant coefficients, so ma[t]
is a causal convolution of x with the impulse response h[d] = (A^d c)[0]
plus an initial-state term.  |eig(A)| = sqrt(0.7) ~ 0.8367, so h decays
below 4e-11 by d = 128: a 128-tap truncated convolution is exact to
working precision.  Per 128-step time chunk:
    ma_chunk[i] = T0 @ x_chunk[i] + T1 @ x_chunk[i-1]
with lower/upper-triangular Toeplitz matrices T0/T1 (and an exact
special-cased first-chunk matrix TF that folds in the initial state).
These run as matmuls on the tensor engine with time on the contraction
axis (partitions); channels ride the free axis.

Precision/bandwidth tradeoff: the kernel is HBM-bandwidth-bound, and the
target tolerance (rel_err < 2e-2) leaves ~5x headroom over bf16
quantization (measured end-to-end max_rel ~ 6e-3).  So x is downcast to
bf16 on the host, the device moves bf16 in/out (24 MB/core instead of
48 MB), accumulates matmuls in fp32 PSUM, and outputs are upcast to f32
on the host after the gather.  HBM floor: 24 MB / ~358 GB/s ~ 70 us.

DMA layout: transfers must cover long *sequential* DRAM spans per DMA to
reach the per-NC HBM roofline (b-interleaved chunk layouts measured ~13%
slower).  Each DMA covers NB consecutive time chunks of one batch row,
tile [t=128, j=NB, c=512].

Sharding: fully data-parallel over batch, 4 batches per core x 8 cores.
"""

import numpy as np
import ml_dtypes

ALPHA = 0.3
BETA = 0.3
B, T, C = 32, 2048, 512
N_CORES = 8
B_LOCAL = B // N_CORES  # 4
L = 128                 # chunk length == conv taps
N_CHUNKS = T // L       # 16
NB = 4                  # chunks per DMA macro-tile (512 KB bf16 span)

BF16 = ml_dtypes.bfloat16


def _build_matrices():
    A = np.array([[1 - ALPHA, 1 - ALPHA],
                  [-ALPHA * BETA, 1 - ALPHA * BETA]], dtype=np.float64)
    c = np.array([ALPHA, ALPHA * BETA], dtype=np.float64)

    # impulse response h[d] = (A^d c)[0], d = 0..2L-1
    hh = np.zeros(2 * L)
    v = c.copy()
    for d in range(2 * L):
        hh[d] = v[0]
        v = A @ v

    # initial-state response p[j], q[j] = (A^j)[0, :]
    p = np.zeros(L)
    q = np.zeros(L)
    M = np.eye(2)
    for j in range(L):
        p[j] = M[0, 0]
        q[j] = M[0, 1]
        M = A @ M

    T0 = np.zeros((L, L))
    for j in range(L):
        T0[j, :j + 1] = hh[j::-1]          # T0[j, k] = h[j - k], k <= j
    T1 = np.zeros((L, L))
    for j in range(L):
        for k in range(j + 1, L):
            T1[j, k] = hh[L + j - k]       # cross-chunk taps, distance < L
    TF = T0.copy()                          # first chunk: exact init state
    TF[0, :] = 0.0
    TF[0, 0] = 1.0                          # ma[0] = x[0]
    for j in range(1, L):
        TF[j, 0] = p[j] - q[j]             # coeff on x[0]
        TF[j, 1] = hh[j - 1] + q[j]        # coeff on x[1]

    # matmul computes lhsT.T @ rhs -> pass the transpose as the stationary op
    tobf = lambda m: np.ascontiguousarray(m.T).astype(BF16)
    return tobf(T0), tobf(T1), tobf(TF)


_NC_CACHE = {}


def _build_nc(n_iter=1, nb=NB, xbufs=3, obufs=4):
    key = (n_iter, nb, xbufs, obufs)
    if key in _NC_CACHE:
        return _NC_CACHE[key]

    import concourse.bacc as bacc
    import concourse.mybir as mybir
    import concourse.tile as tile

    bf16 = mybir.dt.bfloat16
    f32 = mybir.dt.float32
    nc = bacc.Bacc("TRN2", target_bir_lowering=False, debug=False)

    x = nc.dram_tensor("x", [B_LOCAL, T, C], bf16, kind="ExternalInput")
    res = nc.dram_tensor("res", [B_LOCAL, T, C], bf16, kind="ExternalOutput")
    ma = nc.dram_tensor("ma", [B_LOCAL, T, C], bf16, kind="ExternalOutput")

    w0t_np, w1t_np, wft_np = _build_matrices()
    w0d = nc.inline_tensor(w0t_np, name="w0T")
    w1d = nc.inline_tensor(w1t_np, name="w1T")
    wfd = nc.inline_tensor(wft_np, name="wfT")

    xap = x.ap()
    res_ap = res.ap()
    ma_ap = ma.ap()
    n_macro = N_CHUNKS // nb

    def bspan(ap, b, i):
        # one batch row's nb consecutive time chunks: a sequential DRAM span
        # viewed as [L(time, partitions), nb(chunk), C]
        return ap[b, i * nb * L:(i + 1) * nb * L, :].rearrange(
            "(j t) c -> t j c", t=L)

    with tile.TileContext(nc) as tc:
        with (
            tc.tile_pool(name="weights", bufs=1) as wpool,
            tc.tile_pool(name="xin", bufs=xbufs) as xpool,
            tc.tile_pool(name="maout", bufs=obufs) as mapool,
            tc.tile_pool(name="resout", bufs=obufs) as respool,
            tc.tile_pool(name="psum", bufs=4, space="PSUM") as pspool,
        ):
            w0 = wpool.tile([L, L], bf16, tag="w0")
            nc.sync.dma_start(w0[:], w0d[:])
            w1 = wpool.tile([L, L], bf16, tag="w1")
            nc.sync.dma_start(w1[:], w1d[:])
            wf = wpool.tile([L, L], bf16, tag="wf")
            nc.sync.dma_start(wf[:], wfd[:])

            for _rep in range(n_iter):
                x_prev = [None] * B_LOCAL
                for i in range(n_macro):
                    for b in range(B_LOCAL):
                        xt = xpool.tile([L, nb, C], bf16, tag=f"x{b}")
                        nc.sync.dma_start(xt[:], bspan(xap, b, i))

                        ma_t = mapool.tile([L, nb, C], bf16, tag="ma")
                        res_t = respool.tile([L, nb, C], bf16, tag="res")
                        # 2-bank PSUM tiles; PSUM-reading copy on DVE at
                        # [128, 1024] granularity; the sub then runs
                        # all-SBUF bf16 (2x DVE mode eligible).
                        for g in range(nb // 2):
                            ps = pspool.tile([L, 2, C], f32, tag="ps")
                            for k in range(2):
                                j = 2 * g + k
                                if i == 0 and j == 0:
                                    nc.tensor.matmul(ps[:, k, :], wf[:],
                                                     xt[:, 0, :],
                                                     start=True, stop=True)
                                else:
                                    xp, jp = ((xt, j - 1) if j > 0
                                              else (x_prev[b], nb - 1))
                                    nc.tensor.matmul(ps[:, k, :], w1[:],
                                                     xp[:, jp, :],
                                                     start=True, stop=False)
                                    nc.tensor.matmul(ps[:, k, :], w0[:],
                                                     xt[:, j, :],
                                                     start=False, stop=True)
                            jsl = slice(2 * g, 2 * g + 2)
                            flat = lambda ap_: ap_.rearrange("t k c -> t (k c)")
                            nc.vector.tensor_copy(
                                flat(ma_t[:, jsl, :]), flat(ps[:]))
                            nc.vector.tensor_sub(
                                flat(res_t[:, jsl, :]),
                                flat(xt[:, jsl, :]),
                                flat(ma_t[:, jsl, :]))

                        # out-DMAs on the ACT HWDGE queue: their sem waits
                        # would head-of-line-block the SP queue's input DMAs
                        nc.scalar.dma_start(bspan(ma_ap, b, i), ma_t[:])
                        nc.scalar.dma_start(bspan(res_ap, b, i), res_t[:])
                        x_prev[b] = xt

    nc.compile()
    _NC_CACHE[key] = nc
    return nc


def kernel(x):
    x = np.asarray(x)
    assert x.shape == (B, T, C), x.shape
    xb = np.ascontiguousarray(x.astype(BF16))

    from concourse import bass_utils

    nc = _build_nc()
    in_maps = [{"x": xb[i * B_LOCAL:(i + 1) * B_LOCAL]} for i in range(N_CORES)]
    out = bass_utils.run_bass_kernel_spmd(nc, in_maps, core_ids=list(range(N_CORES)))
    res = np.concatenate(
        [np.asarray(out.results[i]["res"]).astype(np.float32)
         for i in range(N_CORES)], axis=0)
    ma = np.concatenate(
        [np.asarray(out.results[i]["ma"]).astype(np.float32)
         for i in range(N_CORES)], axis=0)
    return res, ma
